# revision 17
# baseline (speedup 1.0000x reference)
"""JSD loss kernel for Trainium2 (8 NeuronCores, row-sharded SPMD).

loss[r] = beta*KL(P||M) + (1-beta)*KL(Q||M), beta=0.5, M=(P+Q)/2
        = 0.5*sum_v[ p*lp + q*lq ] - sum_v[ m*log(m) ]

Per-chunk dataflow (per core: 256 rows x 32000 vocab, chunks of 2048):
  DMA :  lp,lq chunk -> SBUF (contiguous [lp|lq] tile)
  ACT :  p = Exp(lp), q = Exp(lq)                (fp32, one shared table)
  PE  :  mt = p + q  (float32r identity matmuls -> PSUM, 1 cyc/row)
  ACT :  logm = Ln(0.5 * mt)  (= log m, scale folds the /2)
  DVE :  STT-AB: ab += sum( (0.5*[lp|lq]) * [p|q] )   = 0.5*(A+B)
         STT-C : c  += sum( (0.5*logm)    * mt )      = sum m*log m
  loss = sum_chunks ab - c
Chunked partial sums keep the big-sum cancellation error ~1e-5.
"""

import sys
from contextlib import ExitStack

import numpy as np

sys.path.insert(0, "/opt/trn_rl_repo")

N, V = 2048, 32000
NCORES = 8
R = N // NCORES  # rows per core = 256
P = 128  # partitions
NBLK = R // P  # row blocks per core = 2
# 32000 cols per block. Small leading chunks on the FIRST block prime the
# DMA->ACT->PE->ACT->DVE pipeline (DVE idles ~8.5us at startup otherwise);
# small trailing chunks on the LAST block shorten the drain. Middle block
# boundaries overlap, so other blocks use uniform 2048 chunks.
CHUNKS_MID = [2048] * 15 + [1280]
CHUNKS_FIRST = [256, 256, 512, 1024] + [2048] * 14 + [1280]
CHUNKS_LAST = [2048] * 15 + [768, 512]
CHUNKS_ONLY = [256, 256, 512, 1024] + [2048] * 13 + [2048, 768, 512]
LN2 = 0.6931471805599453

_CACHE = {}


def _build_program():
    import concourse.bacc as bacc
    import concourse.tile as tile
    from concourse import mybir

    nc = bacc.Bacc(
        "TRN2",
        target_bir_lowering=False,
        debug=False,
        enable_asserts=False,
        num_devices=1,
    )
    lp_d = nc.dram_tensor("log_p", [R, V], mybir.dt.float32, kind="ExternalInput")
    lq_d = nc.dram_tensor("log_q", [R, V], mybir.dt.float32, kind="ExternalInput")
    id_d = nc.dram_tensor("ident", [P, P], mybir.dt.float32, kind="ExternalInput")
    out_d = nc.dram_tensor("loss", [R, 1], mybir.dt.float32, kind="ExternalOutput")

    lp = lp_d.ap()
    lq = lq_d.ap()
    out = out_d.ap()

    fp32 = mybir.dt.float32
    f32r = mybir.dt.float32r
    Exp = mybir.ActivationFunctionType.Exp
    Ln = mybir.ActivationFunctionType.Ln
    mult = mybir.AluOpType.mult

    with tile.TileContext(nc) as tc, ExitStack() as ctx:
        const = ctx.enter_context(tc.tile_pool(name="const", bufs=1))
        loads = ctx.enter_context(tc.tile_pool(name="loads", bufs=6))
        acts = ctx.enter_context(tc.tile_pool(name="acts", bufs=3))
        logms = ctx.enter_context(tc.tile_pool(name="logms", bufs=2))
        scr = ctx.enter_context(tc.tile_pool(name="scr", bufs=2))
        parts = ctx.enter_context(tc.tile_pool(name="parts", bufs=2))
        outs = ctx.enter_context(tc.tile_pool(name="outs", bufs=2))
        psum = ctx.enter_context(tc.tile_pool(name="psum", bufs=2, space="PSUM"))

        # fp32r tile: 0.0/1.0 are exact in the rounded format, so a raw DMA
        # of fp32 bits is valid fp32r data
        ident_sb = const.tile([P, P], f32r)
        # scalar queue: keeps the SP queue free so chunk-0's lp DMA issues
        # immediately (ACT's first exp waits on that DMA anyway)
        nc.scalar.dma_start(out=ident_sb[:], in_=id_d.ap().bitcast(f32r))
        ident_r = ident_sb[:]

        for b in range(NBLK):
            if NBLK == 1:
                chunks = CHUNKS_ONLY
            elif b == 0:
                chunks = CHUNKS_FIRST
            elif b == NBLK - 1:
                chunks = CHUNKS_LAST
            else:
                chunks = CHUNKS_MID
            nch = len(chunks)
            r0 = b * P
            ab_parts = parts.tile([P, nch], fp32, tag="abp")
            c_parts = parts.tile([P, nch], fp32, tag="cp")
            for i, C in enumerate(chunks):
                c0 = sum(chunks[:i])
                lplq = loads.tile([P, 2 * 2048], fp32, tag="lplq")
                # fp32r so the Exp output is rounded for the fp32r matmul
                # (walrus birverifier requires rounded producers); the DVE
                # STT reads it bitcast back to fp32 — same bits.
                pq = acts.tile([P, 2 * 2048], f32r, tag="pq")
                # lp on the SP DMA queue, lq on the idle Pool (gpsimd) queue:
                # the two transfers overlap, keeping aggregate DMA below the
                # ~360 GB/s physical per-core ceiling while un-bottlenecking
                # the stream (single queue models only ~332 GB/s).
                nc.sync.dma_start(out=lplq[:, 0:C], in_=lp[r0 : r0 + P, c0 : c0 + C])
                nc.gpsimd.dma_start(
                    out=lplq[:, C : 2 * C], in_=lq[r0 : r0 + P, c0 : c0 + C]
                )
                # one fused Exp over the contiguous [lp|lq] tile
                nc.scalar.activation(
                    out=pq[:, 0 : 2 * C], in_=lplq[:, 0 : 2 * C], func=Exp
                )
                # mt = p + q  (fp32r identity matmuls accumulate into PSUM)
                m_ps = psum.tile([P, 2048], fp32, tag="m")
                pq_r = pq[:]
                for j0 in range(0, C, 512):
                    w = min(512, C - j0)
                    nc.tensor.matmul(
                        out=m_ps[:, j0 : j0 + w],
                        lhsT=ident_r,
                        rhs=pq_r[:, j0 : j0 + w],
                        start=True,
                        stop=False,
                    )
                    nc.tensor.matmul(
                        out=m_ps[:, j0 : j0 + w],
                        lhsT=ident_r,
                        rhs=pq_r[:, C + j0 : C + j0 + w],
                        start=False,
                        stop=True,
                    )
                # logm = Ln(0.5*mt) = log(m)
                logm = logms.tile([P, 2048], fp32, tag="logm")
                nc.scalar.activation(
                    out=logm[:, 0:C], in_=m_ps[:, 0:C], func=Ln, scale=0.5
                )

                junk = scr.tile([P, 2 * 2048], fp32, tag="junk")
                # AB: sum over both halves of (0.5*[lp|lq])*[p|q] = 0.5*(A+B)
                nc.vector.scalar_tensor_tensor(
                    out=junk[:, 0 : 2 * C],
                    in0=lplq[:, 0 : 2 * C],
                    scalar=0.5,
                    in1=pq[:].bitcast(fp32)[:, 0 : 2 * C],
                    op0=mult,
                    op1=mult,
                    accum_out=ab_parts[:, i : i + 1],
                )
                # C: sum (0.5*logm)*mt = sum m*log m
                nc.vector.scalar_tensor_tensor(
                    out=junk[:, 0:C],
                    in0=logm[:, 0:C],
                    scalar=0.5,
                    in1=m_ps[:, 0:C],
                    op0=mult,
                    op1=mult,
                    accum_out=c_parts[:, i : i + 1],
                )
            d_parts = parts.tile([P, nch], fp32, tag="dp")
            nc.vector.tensor_sub(d_parts[:], ab_parts[:], c_parts[:])
            loss_b = outs.tile([P, 1], fp32)
            nc.vector.reduce_sum(
                out=loss_b[:], in_=d_parts[:], axis=mybir.AxisListType.X
            )
            nc.sync.dma_start(out=out[r0 : r0 + P, :], in_=loss_b[:])

    # bacc's insert_act_table_loads picks the FIRST act-table containing each
    # func (Exp->exp_and_others, Ln->natural_log), thrashing 1283ns reloads on
    # every Exp<->Ln alternation. Mask the funcs of tables that don't contain
    # BOTH Exp and Ln (preserving list order, so the emitted act_func_set_id
    # still indexes act_info.json correctly) to force the combined
    # natural_log_exp_and_others table: one load for the whole kernel.
    import concourse.bacc as bacc_mod

    orig_tables = bacc_mod.get_activation_tables

    def _combined_tables(arch):
        t = orig_tables(arch)
        return {
            name: (funcs if (Exp in funcs and Ln in funcs) else set())
            for name, funcs in t.items()
        }

    bacc_mod.get_activation_tables = _combined_tables
    try:
        nc.compile()
    finally:
        bacc_mod.get_activation_tables = orig_tables
    return nc


def _get_program():
    if "nc" not in _CACHE:
        _CACHE["nc"] = _build_program()
    return _CACHE["nc"]


def kernel(log_q: np.ndarray, log_p: np.ndarray, _trace: bool = False):
    from concourse.bass_utils import run_bass_kernel_spmd

    log_q = np.ascontiguousarray(np.asarray(log_q, dtype=np.float32))
    log_p = np.ascontiguousarray(np.asarray(log_p, dtype=np.float32))
    assert log_q.shape == (N, V) and log_p.shape == (N, V)

    nc = _get_program()
    ident = np.eye(P, dtype=np.float32)
    in_maps = []
    for c in range(NCORES):
        sl = slice(c * R, (c + 1) * R)
        in_maps.append({"log_p": log_p[sl], "log_q": log_q[sl], "ident": ident})
    res = run_bass_kernel_spmd(
        nc, in_maps, core_ids=list(range(NCORES)), trace=_trace
    )
    _CACHE["last_results"] = res
    outs = [res.results[c]["loss"].reshape(R) for c in range(NCORES)]
    return np.concatenate(outs, axis=0).astype(np.float32)
